# revision 18
# baseline (speedup 1.0000x reference)
"""MoE (top-2 of 8 experts, B=8192, D=2048) on 8 Trainium2 NeuronCores.

Strategy (expert-parallel, per sharding hint): the host computes the gate
softmax + top-2 routing (float64 numpy; rank-2/3 margins are ~3e-5 so the
selection matches any f32 reference platform), dispatches each token's rows
to its experts' cores, and each core computes
    y_e = relu(x_e @ W[e].T + b[e]) * gate_scale
for its gathered tokens as an fp16 tiled matmul on the PE array.  The host
then scatter-adds the (at most 2) expert contributions per token.

Device schedule (v2, from the ntff trace of v1): the PE stream runs at the
N=512 issue bound (~217ns/MM; LDWEIGHTS is hidden by the PE reorder
window), so the only recoverable time is head/tail: v1 idled 10us waiting
for wt0/xt0 (bias hogged the SP ring) which also re-throttled the HAM
clock.  v2 splits wt0 across the SP+GPSIMD rings in 4 chunks, keeps xt
alone on the ACT ring, defers bias/scale behind wt0, moves y stores to the
GPSIMD ring, and runs phase 2 one psum bank at a time (n-sequential) so
the final epilogue tail is one tile, not three.
"""

import math

import numpy as np

B, D, E, TOP_K = 8192, 2048, 8, 2
N_CORES = 8
P = 128
KD = D // P  # 16 contraction chunks
NT = 4
NSZ = D // NT  # 512 output columns per psum tile
WARMUP = 14

_F16 = np.float16

_nc_cache = {}


def _routing(x, Wg, bg):
    """Gate softmax + top-2 in float64; returns (idx [B,2] int, vals [B,2] f32)."""
    logits = x.astype(np.float64) @ Wg.astype(np.float64).T + bg.astype(np.float64)
    logits -= logits.max(-1, keepdims=True)
    eL = np.exp(logits)
    gate = eL / eL.sum(-1, keepdims=True)
    order = np.argsort(-gate, axis=-1, kind="stable")
    idx = order[:, :TOP_K]
    vals = np.take_along_axis(gate, idx, -1).astype(np.float32)
    return idx, vals


def _build(m_tiles):
    """Build + compile the per-core Bass kernel for C = m_tiles*128 tokens."""
    import concourse.mybir as mybir
    import concourse.tile as tile
    from concourse import bacc

    nc = bacc.Bacc("TRN2", target_bir_lowering=False)
    C = m_tiles * P
    xt = nc.dram_tensor("xt", [P, m_tiles, KD, P], mybir.dt.float16, kind="ExternalInput")
    wt = nc.dram_tensor("wt", [P, NT, KD, NSZ], mybir.dt.float16, kind="ExternalInput")
    bias = nc.dram_tensor("bias", [P, D], mybir.dt.float32, kind="ExternalInput")
    scale = nc.dram_tensor("scale", [P, m_tiles], mybir.dt.float32, kind="ExternalInput")
    y = nc.dram_tensor("y", [C, D], mybir.dt.float16, kind="ExternalOutput")

    with tile.TileContext(nc) as tc:
        with (
            tc.tile_pool(name="wp", bufs=1) as wp,
            tc.tile_pool(name="xp", bufs=1) as xp,
            tc.tile_pool(name="cp", bufs=1) as cp,
            tc.tile_pool(name="op", bufs=8) as op_,
            tc.tile_pool(name="pp", bufs=7, space="PSUM") as pp,
            tc.tile_pool(name="pw", bufs=1, space="PSUM") as pw,
        ):
            def epilogue(ps, bias_sb, scale_sb, m, n):
                ot = op_.tile([P, NSZ], mybir.dt.float16, tag="ot", name="ot")
                nc.vector.tensor_tensor(
                    ot[:], ps[:], bias_sb[:, n * NSZ:(n + 1) * NSZ], mybir.AluOpType.add
                )
                # relu((z+b)*s) with s>=0, via one DVE tensor_scalar (mult, max 0)
                nc.vector.tensor_scalar(
                    ot[:], ot[:], scale_sb[:, m:m + 1], 0.0,
                    mybir.AluOpType.mult, mybir.AluOpType.max,
                )
                # y stores ride SP; their epilogue data-deps also pace the
                # SP ring so wt2/wt3 (emitted mid-phase-1) can't starve xt.
                nc.sync.dma_start(y[m * P:(m + 1) * P, n * NSZ:(n + 1) * NSZ], ot[:])

            def body():
                # Warm-tile memset first on gpsimd so PE warmup starts ASAP.
                warm = cp.tile([P, 640], mybir.dt.float16, tag="warm", name="warm")
                nc.gpsimd.memset(warm[:], 0.0)

                # Head schedule (rings are per-queue FIFO; SP ~0.42MB/us, ACT
                # ~0.26MB/us, transfers from ~8us).  Everything the m0 sweep
                # touches is ordered so it lands before its ~216ns/MM
                # consumption time: xt0 halves + wt0 chunks q0-q5 lead SP;
                # q6/q7 lead ACT.  (gpsimd DMA is software-DGE at ~40GB/s --
                # never put bulk traffic there.)
                wt0c = []
                for c in range(8):
                    t = wp.tile([P, 2, NSZ], mybir.dt.float16, tag=f"wt0c{c}", name=f"wt0c{c}")
                    wt0c.append(t)
                xts = [None] * m_tiles
                for m in range(1, m_tiles):
                    xts[m] = xp.tile([P, KD, P], mybir.dt.float16, tag=f"xt{m}", name=f"xt_sb{m}")
                xt0a = xp.tile([P, KD // 2, P], mybir.dt.float16, tag="xt0a", name="xt_sb0a")
                xt0b = xp.tile([P, KD // 2, P], mybir.dt.float16, tag="xt0b", name="xt_sb0b")

                def x_op(m, kd):
                    if m == 0:
                        return xt0a[:, kd] if kd < KD // 2 else xt0b[:, kd - KD // 2]
                    return xts[m][:, kd]

                bias_sb = cp.tile([P, D], mybir.dt.float32, tag="bias", name="bias_sb")
                scale_sb = cp.tile([P, m_tiles], mybir.dt.float32, tag="scale", name="scale_sb")
                wts = [None] * NT
                for n in range(1, NT):
                    wts[n] = wp.tile([P, KD, NSZ], mybir.dt.float16, tag=f"wt{n}", name=f"wt_sb{n}")

                # SP ring, in FIFO consumption order.
                nc.sync.dma_start(xt0a[:], xt[:, 0, 0:KD // 2])
                nc.sync.dma_start(xt0b[:], xt[:, 0, KD // 2:KD])
                for c in range(6):
                    nc.sync.dma_start(wt0c[c][:], wt[:, 0, 2 * c:2 * c + 2])
                if m_tiles > 1:
                    nc.sync.dma_start(xts[1][:], xt[:, 1])
                nc.sync.dma_start(bias_sb[:], bias[:])
                nc.sync.dma_start(scale_sb[:], scale[:])
                for m in range(7, m_tiles, 2):
                    nc.sync.dma_start(xts[m][:], xt[:, m])

                # ACT ring: last two wt0 chunks, then the even token tiles
                # (and tiles 3/5, which SP has no room for early).
                nc.scalar.dma_start(wt0c[6][:], wt[:, 0, 12:14])
                nc.scalar.dma_start(wt0c[7][:], wt[:, 0, 14:16])
                for m in [2, 3, 4, 5, 6]:
                    if m < m_tiles:
                        nc.scalar.dma_start(xts[m][:], xt[:, m])
                for m in range(8, m_tiles, 2):
                    nc.scalar.dma_start(xts[m][:], xt[:, m])

                # PE warmup while the first DMAs are in flight: garbage
                # matmuls un-throttle the HAM clock gate (1.2->2.4GHz).
                wps = pw.tile([P, NSZ], mybir.dt.float32, tag="warmps", name="warmps")
                for _w in range(WARMUP):
                    nc.tensor.matmul(wps[:], warm[:, 0:P], warm[:, P:P + NSZ],
                                     start=True, stop=True)

                def filler(k):
                    for _ in range(k):
                        nc.tensor.matmul(wps[:], warm[:, 0:P], warm[:, P:P + NSZ],
                                         start=True, stop=True)

                # Phase 1: n=0 sweep over all m-tiles; the remaining weight
                # and token DMAs hide under this ~60us sweep.  wt1/wt2 are
                # emitted after y-store dma_starts whose data-deps stall the
                # SP engine until phase-1 progress catches up -- pacing the
                # prefetch so it can't compete with the live xt stream.  Two
                # filler matmuls after m0 absorb arrival jitter without
                # idling the PE (idle >2us re-throttles the HAM clock).
                for m in range(m_tiles):
                    ps = pp.tile([P, NSZ], mybir.dt.float32, tag="ps", name="ps")
                    for kd in range(KD):
                        nc.tensor.matmul(
                            ps[:], x_op(m, kd), wt0c[kd // 2][:, kd % 2],
                            start=(kd == 0), stop=(kd == KD - 1),
                        )
                    epilogue(ps, bias_sb, scale_sb, m, 0)
                    if m == 0:
                        filler(2)
                        nc.sync.dma_start(wts[1][:], wt[:, 1])
                    elif m == 1:
                        filler(1)
                    if m == min(11, m_tiles - 1):
                        nc.sync.dma_start(wts[2][:], wt[:, 2])

                # Phase 2: n-outer so wt3 isn't needed until ~2/3 through;
                # its prefetch is paced deep into phase 2 by a y-store dep.
                for n in range(1, NT):
                    for m in range(m_tiles):
                        ps = pp.tile([P, NSZ], mybir.dt.float32, tag="ps", name="ps")
                        for kd in range(KD):
                            nc.tensor.matmul(
                                ps[:], x_op(m, kd), wts[n][:, kd],
                                start=(kd == 0), stop=(kd == KD - 1),
                            )
                        epilogue(ps, bias_sb, scale_sb, m, n)
                        if n == 1 and m == min(8, m_tiles - 1):
                            nc.sync.dma_start(wts[3][:], wt[:, 3])

            body()

    nc.compile()
    return nc


def _get_nc(m_tiles):
    if m_tiles not in _nc_cache:
        _nc_cache[m_tiles] = _build(m_tiles)
    return _nc_cache[m_tiles]


def _prep_inputs(x, W, b, idx, vals):
    """Per-core input maps: blocked fp16 xT/wT layouts + bias/scale tiles."""
    in_maps = []
    token_lists = []
    counts = []
    for e in range(E):
        tok = np.where((idx == e).any(axis=1))[0]
        token_lists.append(tok)
        counts.append(len(tok))
    c_max = max(counts)
    m_tiles = max(1, math.ceil(c_max / P))
    C = m_tiles * P

    for e in range(E):
        tok = token_lists[e]
        cnt = len(tok)
        Xp = np.zeros((C, D), dtype=_F16)
        Xp[:cnt] = x[tok].astype(_F16)
        xt_np = np.ascontiguousarray(
            Xp.reshape(m_tiles, P, KD, P).transpose(3, 0, 2, 1)
        )
        wt_np = np.ascontiguousarray(
            W[e].astype(_F16).reshape(NT, NSZ, KD, P).transpose(3, 0, 2, 1)
        )
        bias_np = np.ascontiguousarray(np.broadcast_to(b[e], (P, D)).astype(np.float32))
        s_tok = np.zeros(C, dtype=np.float32)
        for k in range(TOP_K):
            sel = idx[tok, k] == e
            s_tok[:cnt][sel] = vals[tok[sel], k]
        scale_np = np.ascontiguousarray(s_tok.reshape(m_tiles, P).T)
        in_maps.append({"xt": xt_np, "wt": wt_np, "bias": bias_np, "scale": scale_np})
    return in_maps, token_lists, counts, m_tiles


def kernel(x, W, b, Wg, bg):
    from concourse.bass_utils import run_bass_kernel_spmd

    x = np.asarray(x, dtype=np.float32)
    W = np.asarray(W, dtype=np.float32)
    b = np.asarray(b, dtype=np.float32)
    Wg = np.asarray(Wg, dtype=np.float32)
    bg = np.asarray(bg, dtype=np.float32)

    idx, vals = _routing(x, Wg, bg)
    in_maps, token_lists, counts, m_tiles = _prep_inputs(x, W, b, idx, vals)
    nc = _get_nc(m_tiles)
    res = run_bass_kernel_spmd(nc, in_maps, core_ids=list(range(N_CORES)))

    out = np.zeros((B, D), dtype=np.float32)
    for e in range(E):
        ye = res.results[e]["y"]
        out[token_lists[e]] += ye[:counts[e]].astype(np.float32)
    return out


# revision 21
# speedup vs baseline: 1.0238x; 1.0238x over previous
"""MoE (top-2 of 8 experts, B=8192, D=2048) on 8 Trainium2 NeuronCores.

Strategy (expert-parallel, per sharding hint): the host computes the gate
softmax + top-2 routing (float64 numpy; rank-2/3 margins are ~3e-5 so the
selection matches any f32 reference platform), dispatches each token's rows
to its experts' cores, and each core computes
    y_e = relu(x_e @ W[e].T + b[e]) * gate_scale
for its gathered tokens as an fp16 tiled matmul on the PE array.  The host
then scatter-adds the (at most 2) expert contributions per token.

Device schedule (v2, from the ntff trace of v1): the PE stream runs at the
N=512 issue bound (~217ns/MM; LDWEIGHTS is hidden by the PE reorder
window), so the only recoverable time is head/tail: v1 idled 10us waiting
for wt0/xt0 (bias hogged the SP ring) which also re-throttled the HAM
clock.  v2 splits wt0 across the SP+GPSIMD rings in 4 chunks, keeps xt
alone on the ACT ring, defers bias/scale behind wt0, moves y stores to the
GPSIMD ring, and runs phase 2 one psum bank at a time (n-sequential) so
the final epilogue tail is one tile, not three.
"""

import math

import numpy as np

B, D, E, TOP_K = 8192, 2048, 8, 2
N_CORES = 8
P = 128
KD = D // P  # 16 contraction chunks
NT = 4
NSZ = D // NT  # 512 output columns per psum tile
WARMUP = 12

_F16 = np.float16

_nc_cache = {}


def _routing(x, Wg, bg):
    """Gate softmax + top-2 in float64; returns (idx [B,2] int, vals [B,2] f32)."""
    logits = x.astype(np.float64) @ Wg.astype(np.float64).T + bg.astype(np.float64)
    logits -= logits.max(-1, keepdims=True)
    eL = np.exp(logits)
    gate = eL / eL.sum(-1, keepdims=True)
    order = np.argsort(-gate, axis=-1, kind="stable")
    idx = order[:, :TOP_K]
    vals = np.take_along_axis(gate, idx, -1).astype(np.float32)
    return idx, vals


def _build(m_tiles):
    """Build + compile the per-core Bass kernel for C = m_tiles*128 tokens."""
    import concourse.mybir as mybir
    import concourse.tile as tile
    from concourse import bacc

    nc = bacc.Bacc("TRN2", target_bir_lowering=False)
    C = m_tiles * P
    xt = nc.dram_tensor("xt", [P, m_tiles, KD, P], mybir.dt.float16, kind="ExternalInput")
    wt = nc.dram_tensor("wt", [P, NT, KD, NSZ], mybir.dt.float16, kind="ExternalInput")
    bias = nc.dram_tensor("bias", [P, D], mybir.dt.float32, kind="ExternalInput")
    scale = nc.dram_tensor("scale", [P, m_tiles], mybir.dt.float32, kind="ExternalInput")
    y = nc.dram_tensor("y", [C, D], mybir.dt.float16, kind="ExternalOutput")

    with tile.TileContext(nc) as tc:
        with (
            tc.tile_pool(name="wp", bufs=1) as wp,
            tc.tile_pool(name="xp", bufs=1) as xp,
            tc.tile_pool(name="cp", bufs=1) as cp,
            tc.tile_pool(name="op", bufs=8) as op_,
            tc.tile_pool(name="pp", bufs=7, space="PSUM") as pp,
            tc.tile_pool(name="pw", bufs=1, space="PSUM") as pw,
        ):
            def epilogue(ps, bias_sb, scale_sb, m, n):
                ot = op_.tile([P, NSZ], mybir.dt.float16, tag="ot", name="ot")
                nc.vector.tensor_tensor(
                    ot[:], ps[:], bias_sb[:, n * NSZ:(n + 1) * NSZ], mybir.AluOpType.add
                )
                # relu((z+b)*s) with s>=0, via one DVE tensor_scalar (mult, max 0)
                nc.vector.tensor_scalar(
                    ot[:], ot[:], scale_sb[:, m:m + 1], 0.0,
                    mybir.AluOpType.mult, mybir.AluOpType.max,
                )
                # y stores ride SP; their epilogue data-deps also pace the
                # SP ring so wt2/wt3 (emitted mid-phase-1) can't starve xt.
                nc.sync.dma_start(y[m * P:(m + 1) * P, n * NSZ:(n + 1) * NSZ], ot[:])

            def body():
                # Warm-tile memset first on gpsimd so PE warmup starts ASAP.
                warm = cp.tile([P, 640], mybir.dt.float16, tag="warm", name="warm")
                nc.gpsimd.memset(warm[:], 0.0)

                # Head schedule (rings are per-queue FIFO; SP ~0.42MB/us, ACT
                # ~0.26MB/us, transfers from ~8us).  Everything the m0 sweep
                # touches is ordered so it lands before its ~216ns/MM
                # consumption time: xt0 halves + wt0 chunks q0-q5 lead SP;
                # q6/q7 lead ACT.  (gpsimd DMA is software-DGE at ~40GB/s --
                # never put bulk traffic there.)
                wt0c = []
                for c in range(8):
                    t = wp.tile([P, 2, NSZ], mybir.dt.float16, tag=f"wt0c{c}", name=f"wt0c{c}")
                    wt0c.append(t)
                xts = [None] * m_tiles
                for m in range(1, m_tiles):
                    xts[m] = xp.tile([P, KD, P], mybir.dt.float16, tag=f"xt{m}", name=f"xt_sb{m}")
                xt0a = xp.tile([P, KD // 2, P], mybir.dt.float16, tag="xt0a", name="xt_sb0a")
                xt0b = xp.tile([P, KD // 2, P], mybir.dt.float16, tag="xt0b", name="xt_sb0b")

                def x_op(m, kd):
                    if m == 0:
                        return xt0a[:, kd] if kd < KD // 2 else xt0b[:, kd - KD // 2]
                    return xts[m][:, kd]

                bias_sb = cp.tile([P, D], mybir.dt.float32, tag="bias", name="bias_sb")
                scale_sb = cp.tile([P, m_tiles], mybir.dt.float32, tag="scale", name="scale_sb")
                wts = [None] * NT
                for n in range(1, NT):
                    wts[n] = wp.tile([P, KD, NSZ], mybir.dt.float16, tag=f"wt{n}", name=f"wt_sb{n}")

                # SP ring, in FIFO consumption order: the m0a half-sweep
                # (kd0-7) needs only xt0a+q0..q3 = 1.25MB.
                nc.sync.dma_start(xt0a[:], xt[:, 0, 0:KD // 2])
                for c in range(4):
                    nc.sync.dma_start(wt0c[c][:], wt[:, 0, 2 * c:2 * c + 2])
                if m_tiles > 1:
                    nc.sync.dma_start(xts[1][:], xt[:, 1])
                nc.sync.dma_start(bias_sb[:], bias[:])
                nc.sync.dma_start(scale_sb[:], scale[:])
                for m in range(5, m_tiles, 2):
                    nc.sync.dma_start(xts[m][:], xt[:, m])

                # ACT ring: xt0b + q4..q7 feed the m0b half-sweep in
                # parallel, then the even token tiles (and xt3).
                nc.scalar.dma_start(xt0b[:], xt[:, 0, KD // 2:KD])
                for c in range(4, 8):
                    nc.scalar.dma_start(wt0c[c][:], wt[:, 0, 2 * c:2 * c + 2])
                for m in [2, 3, 4]:
                    if m < m_tiles:
                        nc.scalar.dma_start(xts[m][:], xt[:, m])
                for m in range(6, m_tiles, 2):
                    nc.scalar.dma_start(xts[m][:], xt[:, m])

                # PE warmup while the first DMAs are in flight: garbage
                # matmuls un-throttle the HAM clock gate (1.2->2.4GHz).
                wps = pw.tile([P, NSZ], mybir.dt.float32, tag="warmps", name="warmps")
                for _w in range(WARMUP):
                    nc.tensor.matmul(wps[:], warm[:, 0:P], warm[:, P:P + NSZ],
                                     start=True, stop=True)

                def filler(k):
                    for _ in range(k):
                        nc.tensor.matmul(wps[:], warm[:, 0:P], warm[:, P:P + NSZ],
                                         start=True, stop=True)

                # Phase 1: n=0 sweep over all m-tiles; the remaining weight
                # and token DMAs hide under this ~60us sweep.  m0 runs as two
                # half-K sweeps (kd0-7 then kd8-15, same psum bank) so real
                # work starts on 1.25MB of input instead of 2.5MB; a filler
                # matmul between them absorbs arrival jitter without idling
                # the PE (idle >2us re-throttles the HAM clock).  wt1/wt2 are
                # emitted after y-store dma_starts whose data-deps stall the
                # SP engine until phase-1 progress catches up -- pacing the
                # prefetch so it can't compete with the live xt stream.
                for m in range(m_tiles):
                    ps = pp.tile([P, NSZ], mybir.dt.float32, tag="ps", name="ps")
                    for kd in range(KD):
                        nc.tensor.matmul(
                            ps[:], x_op(m, kd), wt0c[kd // 2][:, kd % 2],
                            start=(kd == 0), stop=(kd == KD - 1),
                        )
                        if m == 0 and kd == KD // 2 - 1:
                            filler(1)
                    epilogue(ps, bias_sb, scale_sb, m, 0)
                    if m == 0:
                        filler(1)
                        nc.sync.dma_start(wts[1][:], wt[:, 1])
                    elif m == 1:
                        filler(1)
                    if m == min(11, m_tiles - 1):
                        nc.sync.dma_start(wts[2][:], wt[:, 2])

                # Phase 2: n-outer so wt3 isn't needed until ~2/3 through;
                # its prefetch is paced deep into phase 2 by a y-store dep.
                for n in range(1, NT):
                    for m in range(m_tiles):
                        ps = pp.tile([P, NSZ], mybir.dt.float32, tag="ps", name="ps")
                        for kd in range(KD):
                            nc.tensor.matmul(
                                ps[:], x_op(m, kd), wts[n][:, kd],
                                start=(kd == 0), stop=(kd == KD - 1),
                            )
                        epilogue(ps, bias_sb, scale_sb, m, n)
                        if n == 1 and m == min(8, m_tiles - 1):
                            nc.sync.dma_start(wts[3][:], wt[:, 3])

            body()

    nc.compile()
    return nc


def _get_nc(m_tiles):
    if m_tiles not in _nc_cache:
        _nc_cache[m_tiles] = _build(m_tiles)
    return _nc_cache[m_tiles]


def _prep_inputs(x, W, b, idx, vals):
    """Per-core input maps: blocked fp16 xT/wT layouts + bias/scale tiles."""
    in_maps = []
    token_lists = []
    counts = []
    for e in range(E):
        tok = np.where((idx == e).any(axis=1))[0]
        token_lists.append(tok)
        counts.append(len(tok))
    c_max = max(counts)
    m_tiles = max(1, math.ceil(c_max / P))
    C = m_tiles * P

    for e in range(E):
        tok = token_lists[e]
        cnt = len(tok)
        Xp = np.zeros((C, D), dtype=_F16)
        Xp[:cnt] = x[tok].astype(_F16)
        xt_np = np.ascontiguousarray(
            Xp.reshape(m_tiles, P, KD, P).transpose(3, 0, 2, 1)
        )
        wt_np = np.ascontiguousarray(
            W[e].astype(_F16).reshape(NT, NSZ, KD, P).transpose(3, 0, 2, 1)
        )
        bias_np = np.ascontiguousarray(np.broadcast_to(b[e], (P, D)).astype(np.float32))
        s_tok = np.zeros(C, dtype=np.float32)
        for k in range(TOP_K):
            sel = idx[tok, k] == e
            s_tok[:cnt][sel] = vals[tok[sel], k]
        scale_np = np.ascontiguousarray(s_tok.reshape(m_tiles, P).T)
        in_maps.append({"xt": xt_np, "wt": wt_np, "bias": bias_np, "scale": scale_np})
    return in_maps, token_lists, counts, m_tiles


def kernel(x, W, b, Wg, bg):
    from concourse.bass_utils import run_bass_kernel_spmd

    x = np.asarray(x, dtype=np.float32)
    W = np.asarray(W, dtype=np.float32)
    b = np.asarray(b, dtype=np.float32)
    Wg = np.asarray(Wg, dtype=np.float32)
    bg = np.asarray(bg, dtype=np.float32)

    idx, vals = _routing(x, Wg, bg)
    in_maps, token_lists, counts, m_tiles = _prep_inputs(x, W, b, idx, vals)
    nc = _get_nc(m_tiles)
    res = run_bass_kernel_spmd(nc, in_maps, core_ids=list(range(N_CORES)))

    out = np.zeros((B, D), dtype=np.float32)
    for e in range(E):
        ye = res.results[e]["y"]
        out[token_lists[e]] += ye[:counts[e]].astype(np.float32)
    return out
